# revision 1
# baseline (speedup 1.0000x reference)
"""Trainium2 Bass kernel for nn_NetworksPlusCircuit.

Computation: y[b] = circuit(sigmoid(x[b] @ Ws + bs)) for x [1048576, 64].

Key simplification: the SDD circuit f(i) = pos_i*f(i+1) + neg_i*f(i+2) with
neg = 1-l collapses to f(i) == 1 for all i >= 8 (l + (1-l) = 1), so only
labelling columns 1..7 matter (literals 3 and 7 are categorical). The matmul
shrinks to [B,64] @ [64,7] and the circuit to a handful of elementwise ops:
    f7 = l7 + 1
    f6 = l6*l7 + 1
    f5 = l5*(f6-f7) + f7
    f4 = l4*(f5-f6) + f6
    f3 = l3*f4 + f5
    f2 = l2*(f3-f4) + f4
    f1 = l1*(f2-f3) + f3

Sharding: pure data parallel over batch across 8 cores. Per core the host
pre-transposes its x shard to xT2 [128, 65536] (two 64-row d-major halves
stacked on the partition dim) so the PE streams it as the moving operand
against a small stationary block-diagonal weight W2 [128, 16]. Matmul output
lands literal-on-partition in PSUM (two 512-wide chunks per 2-bank PSUM
tile); the scalar engine applies sigmoid (bias folded in as a per-partition
bias) while copying to SBUF, the DVE 32x32 block-transpose flips batch onto
partitions, and the circuit runs as wide strided elementwise ops. The host
inverts the layout permutation on the gathered outputs.

Per-core layout (core-local batch index):
    batch = 65536*h + 32768*t + 4096*v + 1024*g + 512*e + 32*c + r
  h: d-half (stacked on partitions 64*h+d), t: H-tile (2), v: psum-pair
  within H (8), g: partition group = matmul col-tile (4), e: psum free half
  (2), c: 32-block within 512 (16), r: batch-within-32 (32).
  Matmul chunk n = 8*p + 2*g + e (p = 8*t + v) covers w-cols [512n, 512n+512)
  of the half; after the 32x32 transpose, literal j of (h, e, c, r) sits at
  H[32g + r, 1024*v + 512*e + 32*c + 8*h + j].
  F[32g + r, 64*v + 32*e + 2*c + h] = f1;  stored flat to y[t].
"""

import sys

for _p in ("/opt/trn_rl_repo",):
    if _p not in sys.path:
        sys.path.insert(0, _p)

import numpy as np

N_CORES = 8
B_TOTAL = 1048576
D = 64
BC = B_TOTAL // N_CORES      # 131072 batch per core
HALF = BC // 2               # 65536 w-columns per half
NCH = HALF // 512            # 128 matmul chunks of 512 w-cols
NSUP = 8                     # X loads per core
SUP_W = HALF // NSUP         # 8192 w-cols per X load (4 MB)
NPAIR = 16                   # psum pair-tiles per core (8 chunks each)
NH = 4                       # H tiles per core (4 pairs each)
PAIR_W = 4096                # w-cols per psum pair (2 MB X load)


def _split_multiwait_instructions(nc, mybir):
    """This walrus build accepts at most one sync wait per instruction.
    Split any multi-wait instruction into single-wait NoOps on the same
    engine ahead of it (engines execute their queue in order, so semantics
    are unchanged)."""
    n_split = 0
    for fn in nc.m.functions:
        for blk in fn.blocks:
            insts = blk.instructions
            if not any(
                i.sync_info is not None and len(i.sync_info.on_wait) > 1
                for i in insts
            ):
                continue
            out = []
            for inst in insts:
                si = inst.sync_info
                if si is not None and len(si.on_wait) > 1:
                    waits = list(si.on_wait)
                    for k, w in enumerate(waits[:-1]):
                        nop = mybir.InstNoOp(
                            name=f"{inst.name}-sw{k}",
                            engine=inst.engine,
                            ins=[],
                            outs=[],
                            sync_info=mybir.SyncInfo(on_wait=[w], on_update=[]),
                        )
                        out.append(nop)
                        n_split += 1
                    inst.sync_info = mybir.SyncInfo(
                        on_wait=[waits[-1]], on_update=list(si.on_update)
                    )
                out.append(inst)
            blk.instructions = out
    return n_split


def build_program():
    import concourse.bass as bass
    import concourse.mybir as mybir
    from concourse import tile
    from contextlib import ExitStack

    f32 = mybir.dt.float32
    nc = bass.Bass("TRN2")
    xT2 = nc.dram_tensor("xT2", [128, HALF], f32, kind="ExternalInput")
    w2 = nc.dram_tensor("w2", [128, 16], f32, kind="ExternalInput")
    b2 = nc.dram_tensor("b2", [128, 1], f32, kind="ExternalInput")
    y = nc.dram_tensor("y", [NH, 128, 256], f32, kind="ExternalOutput")

    with tile.TileContext(nc) as tc:
        with ExitStack() as ctx:
            wpool = ctx.enter_context(tc.tile_pool(name="wpool", bufs=1))
            xpool = ctx.enter_context(tc.tile_pool(name="xpool", bufs=3))
            spool = ctx.enter_context(tc.tile_pool(name="spool", bufs=4))
            hpool = ctx.enter_context(tc.tile_pool(name="hpool", bufs=2))
            cpool = ctx.enter_context(tc.tile_pool(name="cpool", bufs=1))
            fpool = ctx.enter_context(tc.tile_pool(name="fpool", bufs=2))
            ppool = ctx.enter_context(
                tc.tile_pool(name="ppool", bufs=4, space="PSUM")
            )

            wt = wpool.tile([128, 16], f32)
            nc.sync.dma_start(wt[:], w2[:, :])
            bt = wpool.tile([128, 1], f32)
            nc.sync.dma_start(bt[:], b2[:, :])
            warm = wpool.tile([128, 16], f32)
            nc.scalar.activation(
                warm[:], wt[:], mybir.ActivationFunctionType.Sigmoid,
                bias=bt[:, 0:1], scale=1.0,
            )

            X = None
            for t in range(NH):
                H = hpool.tile([128, 4096], f32)
                for v in range(4):
                    p = 4 * t + v
                    if p % 2 == 0:
                        s = p // 2
                        X = xpool.tile([128, SUP_W], f32, name="X", tag="X")
                        if s == 0:
                            for q4 in range(4):
                                qw = SUP_W // 4
                                nc.sync.dma_start(
                                    X[:, q4 * qw:(q4 + 1) * qw],
                                    xT2[:, q4 * qw:(q4 + 1) * qw],
                                )
                        else:
                            nc.sync.dma_start(
                                X[:], xT2[:, s * SUP_W:(s + 1) * SUP_W]
                            )
                    ps = ppool.tile([128, 1024], f32)
                    for g in range(4):
                        for e in range(2):
                            xoff = 4096 * (p % 2) + 512 * (2 * g + e)
                            nc.tensor.matmul(
                                ps[32 * g:32 * g + 16, 512 * e:512 * (e + 1)],
                                wt[:, :],
                                X[:, xoff:xoff + 512],
                                start=True,
                                stop=True,
                                tile_position=(0, 32 * g),
                            )
                    S = spool.tile([128, 1024], f32)
                    nc.scalar.activation(
                        S[:],
                        ps[:],
                        mybir.ActivationFunctionType.Sigmoid,
                        bias=bt[:, 0:1],
                        scale=1.0,
                    )
                    nc.vector.transpose(H[:, 1024 * v:1024 * (v + 1)], S[:])

                # circuit over H [128, 8192]; literal j of (h-half) at free
                # slot 32*blk + 8*h + j
                H3 = H.rearrange("p (b q) -> p b q", q=32)
                l = lambda j: H3[:, :, j:16:8]  # noqa: E731

                def t3(name):
                    t_ = cpool.tile([128, 256], f32, name=name, tag=name)
                    return t_.rearrange("p (b q) -> p b q", q=2)

                F = fpool.tile([128, 256], f32)
                F3 = F.rearrange("p (b q) -> p b q", q=2)

                f7 = t3("f7")
                nc.vector.tensor_scalar_add(f7, l(6), 1.0)
                pr = t3("pr")
                nc.vector.tensor_mul(pr, l(5), l(6))
                f6 = t3("f6")
                nc.vector.tensor_scalar_add(f6, pr, 1.0)
                d = t3("d")
                nc.vector.tensor_sub(d, f6, f7)
                pr2 = t3("pr2")
                nc.vector.tensor_mul(pr2, l(4), d)
                f5 = t3("f5")
                nc.vector.tensor_add(f5, pr2, f7)
                d2 = t3("d2")
                nc.vector.tensor_sub(d2, f5, f6)
                pr3 = t3("pr3")
                nc.vector.tensor_mul(pr3, l(3), d2)
                f4 = t3("f4")
                nc.vector.tensor_add(f4, pr3, f6)
                pr4 = t3("pr4")
                nc.vector.tensor_mul(pr4, l(2), f4)
                f3 = t3("f3")
                nc.vector.tensor_add(f3, pr4, f5)
                d3 = t3("d3")
                nc.vector.tensor_sub(d3, f3, f4)
                pr5 = t3("pr5")
                nc.vector.tensor_mul(pr5, l(1), d3)
                f2 = t3("f2")
                nc.vector.tensor_add(f2, pr5, f4)
                d4 = t3("d4")
                nc.vector.tensor_sub(d4, f2, f3)
                pr6 = t3("pr6")
                nc.vector.tensor_mul(pr6, l(0), d4)
                nc.vector.tensor_add(F3, pr6, f3)

                nc.scalar.dma_start(y[t], F[:])

    import concourse.mybir as _mybir

    _split_multiwait_instructions(nc, _mybir)
    return nc


def _prep_inputs(x, Ws, bs):
    """Host-side shard + layout prep. Returns per-core input maps."""
    x = np.asarray(x, dtype=np.float32)
    Ws = np.asarray(Ws, dtype=np.float32)
    bs = np.asarray(bs, dtype=np.float32)

    W7 = np.zeros((64, 7), np.float32)
    b7 = np.zeros(7, np.float32)
    for j in range(7):
        W7[:, j] = Ws[j // 4, :, j % 4]
        b7[j] = bs[j // 4, j % 4]
    W2 = np.zeros((128, 16), np.float32)
    W2[0:64, 0:7] = W7
    W2[64:128, 8:15] = W7
    B2 = np.zeros((128, 1), np.float32)
    for g in range(4):
        for h in range(2):
            B2[32 * g + 8 * h:32 * g + 8 * h + 7, 0] = b7

    in_maps = []
    for c in range(N_CORES):
        xc = x[c * BC:(c + 1) * BC]
        xT2 = np.ascontiguousarray(
            xc.reshape(2, HALF, D).transpose(0, 2, 1).reshape(128, HALF)
        )
        in_maps.append({"xT2": xT2, "w2": W2, "b2": B2})
    return in_maps


def _gather_output(results):
    """Invert the device layout; see module docstring for the index map."""
    outs = []
    for c in range(N_CORES):
        yraw = np.asarray(results[c]["y"], dtype=np.float32).reshape(-1)
        yc = (
            yraw.reshape(NH, 4, 32, 4, 2, 16, 2)   # t g r v e c h
            .transpose(6, 0, 3, 1, 4, 5, 2)        # h t v g e c r
            .reshape(BC)
        )
        outs.append(yc)
    return np.concatenate(outs).astype(np.float32)


def run(inputs, trace=False, **run_kwargs):
    """Build, execute on 8 cores, and gather. Returns (y, BassKernelResults)."""
    from concourse.bass_utils import run_bass_kernel_spmd

    nc = build_program()
    in_maps = _prep_inputs(inputs["x"], inputs["Ws"], inputs["bs"])
    res = run_bass_kernel_spmd(
        nc, in_maps, core_ids=list(range(N_CORES)), trace=trace, **run_kwargs
    )
    return _gather_output(res.results), res


def kernel(x, Ws, bs):
    y, _ = run({"x": x, "Ws": Ws, "bs": bs})
    return y


if __name__ == "__main__":
    rng = np.random.default_rng(0)
    x = rng.standard_normal((B_TOTAL, D), dtype=np.float32)
    Ws = (rng.standard_normal((4, 64, 4)) * 0.1).astype(np.float32)
    bs = np.zeros((4, 4), np.float32)
    y = kernel(x, Ws, bs)
    print("kernel ran, y:", y.shape, y.dtype, y[:4])



# revision 2
# speedup vs baseline: 1.6268x; 1.6268x over previous
"""Trainium2 Bass kernel for nn_NetworksPlusCircuit.

Computation: y[b] = circuit(sigmoid(x[b] @ Ws + bs)) for x [1048576, 64].

Circuit simplification (see git history / reference): f(i)=1 for i>=8, so only
labelling columns 1..7 matter and the SDD collapses to

    f7 = l7 + 1
    f6 = l6*l7 + 1
    f5 = l5*(f6-f7) + f7
    f4 = l4*(f5-f6) + f6
    f3 = l3*f4 + f5
    f2 = l2*(f3-f4) + f4
    f1 = l1*(f2-f3) + f3

Sharding: pure data parallel over batch across 8 cores.

Device dataflow (v2 — batch-on-partition, bf16 stream):
  * Host ships x as bf16 xT2 [128, 65536]: two 65536-batch halves stacked on
    the partition dim, d-major (partition 64h+d, free = batch-within-half).
    16 MB/core instead of 32 MB — the kernel is HBM-read-bound, so bf16
    halves the roofline. (bf16 end-to-end max rel err ~4e-3, gate is 2e-2.)
  * Matmul direction is flipped vs v1: the STATIONARY operand is a [128, 128]
    slice of xT2 (128 d-rows x 128 batch-cols, fast weight load at 2 bf16
    cols/cycle) and the MOVING operand is the tiny block-diagonal weight
    w2 [128, 16] (rows 0:64 -> cols 0:7 = W7 for the half-0 batch, rows
    64:128 -> cols 8:15 = W7 for the half-1 batch). Output lands [128 batch
    partitions, 16 literal slots] in PSUM — batch is already on partitions,
    so NO on-chip transpose is needed at all.
  * 32 matmuls fill one PSUM bank [128, 512] = 32 tiles x 16 slots. The
    scalar engine applies sigmoid while DE-INTERLEAVING: two strided
    activations per bank scatter literal j of tile i to contiguous
    per-literal planes in SBUF. After 4 banks (a "quarter"), each literal
    occupies a contiguous [128, 256] plane, so the 17-op circuit runs as
    cheap contiguous DVE ops. f32 planes + f32 circuit preserve precision.
  * Output F [128, 256] per quarter -> y [4, 128, 256]; host inverts the
    layout permutation.

Per-core index map (core-local batch): stationary tile p covers xT2 cols
[128p, 128p+128); batch = 65536h + 128p + m (m = col within tile, h = half).
Fill f = p//32 (i = p%32), quarter q = f//4, tq = f%4. Literal j of (h,i,m)
sits at S_q[m, 256*j + 64*tq + 32*h + i] and F_q[m, 64*tq + 32*h + i];
so y[q][m, 64*tq + 32*h + i] = f1(batch = 65536h + 16384q + 4096tq + 128i + m).
"""

import sys

for _p in ("/opt/trn_rl_repo",):
    if _p not in sys.path:
        sys.path.insert(0, _p)

import numpy as np
import ml_dtypes

N_CORES = 8
B_TOTAL = 1048576
D = 64
BC = B_TOTAL // N_CORES      # 131072 batch per core
HALF = BC // 2               # 65536 xT2 cols (batch-per-half)
NF = 16                      # psum bank fills per core
NQ = 4                       # output quarters (4 fills each)
TPF = 32                     # stationary tiles (matmuls) per fill
CHW = 8192                   # X chunk width (cols); [128, 8192] bf16 = 2 MB


def _split_multiwait_instructions(nc, mybir):
    """This walrus build accepts at most one sync wait per instruction.
    Split any multi-wait instruction into single-wait NoOps on the same
    engine ahead of it (engines execute their queue in order, so semantics
    are unchanged)."""
    n_split = 0
    for fn in nc.m.functions:
        for blk in fn.blocks:
            insts = blk.instructions
            if not any(
                i.sync_info is not None and len(i.sync_info.on_wait) > 1
                for i in insts
            ):
                continue
            out = []
            for inst in insts:
                si = inst.sync_info
                if si is not None and len(si.on_wait) > 1:
                    waits = list(si.on_wait)
                    for k, w in enumerate(waits[:-1]):
                        nop = mybir.InstNoOp(
                            name=f"{inst.name}-sw{k}",
                            engine=inst.engine,
                            ins=[],
                            outs=[],
                            sync_info=mybir.SyncInfo(on_wait=[w], on_update=[]),
                        )
                        out.append(nop)
                        n_split += 1
                    inst.sync_info = mybir.SyncInfo(
                        on_wait=[waits[-1]], on_update=list(si.on_update)
                    )
                out.append(inst)
            blk.instructions = out
    return n_split


def build_program(with_bias=False):
    import concourse.bass as bass
    import concourse.mybir as mybir
    from concourse import tile
    from contextlib import ExitStack

    f32 = mybir.dt.float32
    bf16 = mybir.dt.bfloat16
    SIG = mybir.ActivationFunctionType.Sigmoid
    nc = bass.Bass("TRN2")
    xT2 = nc.dram_tensor("xT2", [128, HALF], bf16, kind="ExternalInput")
    w2 = nc.dram_tensor("w2", [128, 16], bf16, kind="ExternalInput")
    if with_bias:
        ones2 = nc.dram_tensor("ones2", [128, 128], bf16, kind="ExternalInput")
        bias2 = nc.dram_tensor("bias2", [128, 512], bf16, kind="ExternalInput")
    y = nc.dram_tensor("y", [NQ, 128, 256], f32, kind="ExternalOutput")

    with tile.TileContext(nc) as tc:
        with ExitStack() as ctx:
            wpool = ctx.enter_context(tc.tile_pool(name="wpool", bufs=1))
            xpool = ctx.enter_context(tc.tile_pool(name="xpool", bufs=3))
            spool = ctx.enter_context(tc.tile_pool(name="spool", bufs=3))
            cpool = ctx.enter_context(tc.tile_pool(name="cpool", bufs=2))
            fpool = ctx.enter_context(tc.tile_pool(name="fpool", bufs=2))
            ppool = ctx.enter_context(
                tc.tile_pool(name="ppool", bufs=8, space="PSUM")
            )

            wt = wpool.tile([128, 16], bf16)
            nc.sync.dma_start(wt[:], w2[:, :])
            if with_bias:
                onest = wpool.tile([128, 128], bf16)
                nc.sync.dma_start(onest[:], ones2[:, :])
                biast = wpool.tile([128, 512], bf16)
                nc.sync.dma_start(biast[:], bias2[:, :])
            # Prime the sigmoid ACT table during the DMA ramp so the first
            # real activation doesn't pay the table load.
            warm = wpool.tile([128, 16], f32)
            nc.scalar.activation(warm[:], wt[:], SIG)

            X = S = None
            for f in range(NF):
                q, tq = f // 4, f % 4
                if f % 2 == 0:
                    c = f // 2
                    X = xpool.tile([128, CHW], bf16, name="X", tag="X")
                    if c == 0:
                        # split the first chunk so matmuls start early
                        for k in range(4):
                            kw = CHW // 4
                            nc.sync.dma_start(
                                X[:, k * kw:(k + 1) * kw],
                                xT2[:, k * kw:(k + 1) * kw],
                            )
                    else:
                        nc.sync.dma_start(X[:], xT2[:, c * CHW:(c + 1) * CHW])
                if tq == 0:
                    S = spool.tile([128, 7 * 256], f32, name="S", tag="S")

                ps = ppool.tile([128, 512], f32)
                base = (f % 2) * 4096
                if with_bias:
                    nc.tensor.matmul(
                        ps[:, :], onest[:, :], biast[:, :],
                        start=True, stop=False, skip_group_check=True,
                    )
                for i in range(TPF):
                    nc.tensor.matmul(
                        ps[:, 16 * i:16 * i + 16],
                        X[:, base + 128 * i:base + 128 * i + 128],
                        wt[:, :],
                        start=not with_bias,
                        stop=True,
                        skip_group_check=with_bias,
                    )

                # sigmoid + de-interleave: literal j of tile i -> plane j
                psv = ps.rearrange("p (i s) -> p s i", s=16)
                Sv = S.rearrange("p (s u) -> p s u", u=256)
                o = 64 * tq
                nc.scalar.activation(Sv[:, :, o:o + 32], psv[:, 0:7, :], SIG)
                nc.scalar.activation(
                    Sv[:, :, o + 32:o + 64], psv[:, 8:15, :], SIG
                )

                if tq == 3:
                    l = lambda j: S[:, 256 * j:256 * (j + 1)]  # noqa: E731

                    def t(name):
                        return cpool.tile([128, 256], f32, name=name, tag=name)

                    f7 = t("f7")
                    nc.vector.tensor_scalar_add(f7, l(6), 1.0)
                    pr = t("pr")
                    nc.vector.tensor_mul(pr, l(5), l(6))
                    f6 = t("f6")
                    nc.vector.tensor_scalar_add(f6, pr, 1.0)
                    d = t("d")
                    nc.vector.tensor_sub(d, f6, f7)
                    p2 = t("p2")
                    nc.vector.tensor_mul(p2, l(4), d)
                    f5 = t("f5")
                    nc.vector.tensor_add(f5, p2, f7)
                    d2 = t("d2")
                    nc.vector.tensor_sub(d2, f5, f6)
                    p3 = t("p3")
                    nc.vector.tensor_mul(p3, l(3), d2)
                    f4 = t("f4")
                    nc.vector.tensor_add(f4, p3, f6)
                    p4 = t("p4")
                    nc.vector.tensor_mul(p4, l(2), f4)
                    f3 = t("f3")
                    nc.vector.tensor_add(f3, p4, f5)
                    d3 = t("d3")
                    nc.vector.tensor_sub(d3, f3, f4)
                    p5 = t("p5")
                    nc.vector.tensor_mul(p5, l(1), d3)
                    f2 = t("f2")
                    nc.vector.tensor_add(f2, p5, f4)
                    d4 = t("d4")
                    nc.vector.tensor_sub(d4, f2, f3)
                    p6 = t("p6")
                    nc.vector.tensor_mul(p6, l(0), d4)
                    F = fpool.tile([128, 256], f32)
                    nc.vector.tensor_add(F, p6, f3)
                    nc.scalar.dma_start(y[q], F[:])

    import concourse.mybir as _mybir

    _split_multiwait_instructions(nc, _mybir)
    return nc


def _prep_inputs(x, Ws, bs):
    """Host-side shard + layout prep. Returns (per-core input maps, bias?)."""
    x = np.asarray(x, dtype=np.float32)
    Ws = np.asarray(Ws, dtype=np.float32)
    bs = np.asarray(bs, dtype=np.float32)

    W7 = np.zeros((64, 7), np.float32)
    b7 = np.zeros(7, np.float32)
    for j in range(7):
        W7[:, j] = Ws[j // 4, :, j % 4]
        b7[j] = bs[j // 4, j % 4]
    W2 = np.zeros((128, 16), np.float32)
    W2[0:64, 0:7] = W7
    W2[64:128, 8:15] = W7
    W2 = W2.astype(ml_dtypes.bfloat16)

    with_bias = bool(np.any(b7 != 0.0))
    extra = {}
    if with_bias:
        ones2 = np.ones((128, 128), ml_dtypes.bfloat16)
        bias2 = np.zeros((128, 512), np.float32)
        for s in range(7):
            bias2[:, s::16] = b7[s] / 128.0
            bias2[:, 8 + s::16] = b7[s] / 128.0
        extra = {"ones2": ones2, "bias2": bias2.astype(ml_dtypes.bfloat16)}

    in_maps = []
    for c in range(N_CORES):
        xc = x[c * BC:(c + 1) * BC]
        xT2 = np.ascontiguousarray(
            xc.reshape(2, HALF, D).transpose(0, 2, 1).reshape(128, HALF)
        ).astype(ml_dtypes.bfloat16)
        in_maps.append({"xT2": xT2, "w2": W2, **extra})
    return in_maps, with_bias


def _gather_output(results):
    """Invert the device layout; see module docstring for the index map."""
    outs = []
    for c in range(N_CORES):
        yraw = np.asarray(results[c]["y"], dtype=np.float32)
        yc = (
            yraw.reshape(NQ, 128, 4, 2, 32)    # q m tq h i
            .transpose(3, 0, 2, 4, 1)          # h q tq i m
            .reshape(BC)
        )
        outs.append(yc)
    return np.concatenate(outs).astype(np.float32)


def run(inputs, trace=False, **run_kwargs):
    """Build, execute on 8 cores, and gather. Returns (y, BassKernelResults)."""
    from concourse.bass_utils import run_bass_kernel_spmd

    in_maps, with_bias = _prep_inputs(inputs["x"], inputs["Ws"], inputs["bs"])
    nc = build_program(with_bias=with_bias)
    res = run_bass_kernel_spmd(
        nc, in_maps, core_ids=list(range(N_CORES)), trace=trace, **run_kwargs
    )
    return _gather_output(res.results), res


def kernel(x, Ws, bs):
    y, _ = run({"x": x, "Ws": Ws, "bs": bs})
    return y


if __name__ == "__main__":
    rng = np.random.default_rng(0)
    x = rng.standard_normal((B_TOTAL, D), dtype=np.float32)
    Ws = (rng.standard_normal((4, 64, 4)) * 0.1).astype(np.float32)
    bs = np.zeros((4, 4), np.float32)
    y = kernel(x, Ws, bs)
    print("kernel ran, y:", y.shape, y.dtype, y[:4])


# revision 6
# speedup vs baseline: 1.9002x; 1.1681x over previous
"""Trainium2 Bass kernel for nn_NetworksPlusCircuit.

Computation: y[b] = circuit(sigmoid(x[b] @ Ws + bs)) for x [1048576, 64].

Circuit simplification (see git history / reference): f(i)=1 for i>=8, so only
labelling columns 1..7 matter and the SDD collapses to

    f7 = l7 + 1
    f6 = l6*l7 + 1
    f5 = l5*(f6-f7) + f7
    f4 = l4*(f5-f6) + f6
    f3 = l3*f4 + f5
    f2 = l2*(f3-f4) + f4
    f1 = l1*(f2-f3) + f3

Sharding: pure data parallel over batch across 8 cores.

Device dataflow (v2 — batch-on-partition, bf16 stream):
  * Host ships x as bf16 xT2 [128, 65536]: two 65536-batch halves stacked on
    the partition dim, d-major (partition 64h+d, free = batch-within-half).
    16 MB/core instead of 32 MB — the kernel is HBM-read-bound, so bf16
    halves the roofline. (bf16 end-to-end max rel err ~4e-3, gate is 2e-2.)
  * Matmul direction is flipped vs v1: the STATIONARY operand is a [128, 128]
    slice of xT2 (128 d-rows x 128 batch-cols, fast weight load at 2 bf16
    cols/cycle) and the MOVING operand is the tiny block-diagonal weight
    w2 [128, 16] (rows 0:64 -> cols 0:7 = W7 for the half-0 batch, rows
    64:128 -> cols 8:15 = W7 for the half-1 batch). Output lands [128 batch
    partitions, 16 literal slots] in PSUM — batch is already on partitions,
    so NO on-chip transpose is needed at all.
  * 32 matmuls fill one PSUM bank [128, 512] = 32 tiles x 16 slots. The
    scalar engine applies sigmoid while DE-INTERLEAVING: two strided
    activations per bank scatter literal j of tile i to contiguous
    per-literal planes in SBUF. After 4 banks (a "quarter"), each literal
    occupies a contiguous [128, 256] plane, so the 17-op circuit runs as
    cheap contiguous DVE ops. f32 planes + f32 circuit preserve precision.
  * Output F [128, 256] per quarter -> y [4, 128, 256]; host inverts the
    layout permutation.

Per-core index map (core-local batch): stationary tile p covers xT2 cols
[128p, 128p+128); batch = 65536h + 128p + m (m = col within tile, h = half).
Fill f = p//32 (i = p%32), quarter q = f//4, tq = f%4. Literal j of (h,i,m)
sits at S_q[m, 256*j + 64*tq + 32*h + i] and F_q[m, 64*tq + 32*h + i];
so y[q][m, 64*tq + 32*h + i] = f1(batch = 65536h + 16384q + 4096tq + 128i + m).
"""

import sys

for _p in ("/opt/trn_rl_repo",):
    if _p not in sys.path:
        sys.path.insert(0, _p)

import numpy as np
import ml_dtypes

N_CORES = 8
B_TOTAL = 1048576
D = 64
BC = B_TOTAL // N_CORES      # 131072 batch per core
HALF = BC // 2               # 65536 xT2 cols (batch-per-half)
NF = 16                      # psum bank fills per core
NQ = 4                       # output quarters (4 fills each)
TPF = 32                     # stationary tiles (matmuls) per fill
FW = 4096                    # X cols per fill; [128, 4096] bf16 = 1 MB


def _split_multiwait_instructions(nc, mybir):
    """This walrus build accepts at most one sync wait per instruction.
    Split any multi-wait instruction into single-wait NoOps on the same
    engine ahead of it (engines execute their queue in order, so semantics
    are unchanged)."""
    n_split = 0
    for fn in nc.m.functions:
        for blk in fn.blocks:
            insts = blk.instructions
            if not any(
                i.sync_info is not None and len(i.sync_info.on_wait) > 1
                for i in insts
            ):
                continue
            out = []
            for inst in insts:
                si = inst.sync_info
                if si is not None and len(si.on_wait) > 1:
                    waits = list(si.on_wait)
                    for k, w in enumerate(waits[:-1]):
                        nop = mybir.InstNoOp(
                            name=f"{inst.name}-sw{k}",
                            engine=inst.engine,
                            ins=[],
                            outs=[],
                            sync_info=mybir.SyncInfo(on_wait=[w], on_update=[]),
                        )
                        out.append(nop)
                        n_split += 1
                    inst.sync_info = mybir.SyncInfo(
                        on_wait=[waits[-1]], on_update=list(si.on_update)
                    )
                out.append(inst)
            blk.instructions = out
    return n_split


def build_program(with_bias=False):
    import concourse.bass as bass
    import concourse.mybir as mybir
    from concourse import tile
    from contextlib import ExitStack

    f32 = mybir.dt.float32
    bf16 = mybir.dt.bfloat16
    SIG = mybir.ActivationFunctionType.Sigmoid
    nc = bass.Bass("TRN2")
    xT2 = nc.dram_tensor("xT2", [128, HALF], bf16, kind="ExternalInput")
    w2 = nc.dram_tensor("w2", [128, 16], bf16, kind="ExternalInput")
    if with_bias:
        ones2 = nc.dram_tensor("ones2", [128, 128], bf16, kind="ExternalInput")
        bias2 = nc.dram_tensor("bias2", [128, 512], bf16, kind="ExternalInput")
    y = nc.dram_tensor("y", [NQ, 128, 256], f32, kind="ExternalOutput")

    with tile.TileContext(nc) as tc:
        with ExitStack() as ctx:
            wpool = ctx.enter_context(tc.tile_pool(name="wpool", bufs=1))
            xpool = ctx.enter_context(tc.tile_pool(name="xpool", bufs=4))
            spool = ctx.enter_context(tc.tile_pool(name="spool", bufs=3))
            cpool = ctx.enter_context(tc.tile_pool(name="cpool", bufs=2))
            fpool = ctx.enter_context(tc.tile_pool(name="fpool", bufs=2))
            ppool = ctx.enter_context(
                tc.tile_pool(name="ppool", bufs=8, space="PSUM")
            )

            wt = wpool.tile([128, 16], bf16)
            nc.sync.dma_start(wt[:], w2[:, :])
            if with_bias:
                onest = wpool.tile([128, 128], bf16)
                nc.sync.dma_start(onest[:], ones2[:, :])
                biast = wpool.tile([128, 512], bf16)
                nc.sync.dma_start(biast[:], bias2[:, :])
            # Prime the sigmoid ACT table during the DMA ramp so the first
            # real activation doesn't pay the table load.
            warm = wpool.tile([128, 16], f32)
            nc.scalar.activation(warm[:], wt[:], SIG)

            S = None
            for f in range(NF):
                q, tq = f // 4, f % 4
                # one 1 MB X tile per psum fill keeps the matmul stream
                # trailing the DMA stream by ~1 MB instead of a whole chunk
                X = xpool.tile([128, FW], bf16, name="X", tag="X")
                if f == 0:
                    # split the first fill's load so matmuls start early
                    for k in range(4):
                        kw = FW // 4
                        nc.sync.dma_start(
                            X[:, k * kw:(k + 1) * kw],
                            xT2[:, k * kw:(k + 1) * kw],
                        )
                else:
                    nc.sync.dma_start(X[:], xT2[:, f * FW:(f + 1) * FW])
                if tq == 0:
                    S = spool.tile([128, 7 * 256], f32, name="S", tag="S")

                ps = ppool.tile([128, 512], f32)
                if with_bias:
                    nc.tensor.matmul(
                        ps[:, :], onest[:, :], biast[:, :],
                        start=True, stop=False, skip_group_check=True,
                    )
                for i in range(TPF):
                    nc.tensor.matmul(
                        ps[:, 16 * i:16 * i + 16],
                        X[:, 128 * i:128 * i + 128],
                        wt[:, :],
                        start=not with_bias,
                        stop=True,
                        skip_group_check=with_bias,
                    )

                # sigmoid + de-interleave: literal j of tile i -> plane j
                psv = ps.rearrange("p (i s) -> p s i", s=16)
                Sv = S.rearrange("p (s u) -> p s u", u=256)
                o = 64 * tq
                nc.scalar.activation(Sv[:, :, o:o + 32], psv[:, 0:7, :], SIG)
                nc.scalar.activation(
                    Sv[:, :, o + 32:o + 64], psv[:, 8:15, :], SIG
                )

                if tq == 3:
                    # Circuit, refactored for dependency depth 9 (vs 16 for
                    # the naive form) using the identities
                    #   f6-f7      = l7*(l6-1)            =: e1
                    #   f5-f6      = e1*(l5-1)            =: e2
                    #   f3-f4      = (l3-1)*f4 + f5       =: d3
                    #   f2-f3      = d3*(l2-1)            =: d4
                    #   f3         = d3 + f4
                    #   f1         = l1*d4 + f3
                    # scalar_tensor_tensor fuses each (l-1)*t pair.
                    l = lambda j: S[:, 256 * j:256 * (j + 1)]  # noqa: E731
                    A = mybir.AluOpType

                    def t(name):
                        return cpool.tile([128, 256], f32, name=name, tag=name)

                    f7 = t("f7")
                    nc.vector.tensor_scalar_add(f7, l(6), 1.0)
                    pr = t("pr")
                    nc.vector.tensor_mul(pr, l(5), l(6))
                    f6 = t("f6")
                    nc.vector.tensor_scalar_add(f6, pr, 1.0)
                    e1 = t("e1")
                    nc.vector.scalar_tensor_tensor(
                        e1, l(5), -1.0, l(6), A.add, A.mult)
                    p2 = t("p2")
                    nc.vector.tensor_mul(p2, l(4), e1)
                    f5 = t("f5")
                    nc.vector.tensor_add(f5, p2, f7)
                    e2 = t("e2")
                    nc.vector.scalar_tensor_tensor(
                        e2, l(4), -1.0, e1, A.add, A.mult)
                    p3 = t("p3")
                    nc.vector.tensor_mul(p3, l(3), e2)
                    f4 = t("f4")
                    nc.vector.tensor_add(f4, p3, f6)
                    t3 = t("t3")
                    nc.vector.scalar_tensor_tensor(
                        t3, l(2), -1.0, f4, A.add, A.mult)
                    d3 = t("d3")
                    nc.vector.tensor_add(d3, t3, f5)
                    f3 = t("f3")
                    nc.vector.tensor_add(f3, d3, f4)
                    d4 = t("d4")
                    nc.vector.scalar_tensor_tensor(
                        d4, l(1), -1.0, d3, A.add, A.mult)
                    p6 = t("p6")
                    nc.vector.tensor_mul(p6, l(0), d4)
                    F = fpool.tile([128, 256], f32)
                    nc.vector.tensor_add(F, p6, f3)
                    nc.scalar.dma_start(y[q], F[:])

    import concourse.mybir as _mybir

    _split_multiwait_instructions(nc, _mybir)
    return nc


def _prep_inputs(x, Ws, bs):
    """Host-side shard + layout prep. Returns (per-core input maps, bias?)."""
    x = np.asarray(x, dtype=np.float32)
    Ws = np.asarray(Ws, dtype=np.float32)
    bs = np.asarray(bs, dtype=np.float32)

    W7 = np.zeros((64, 7), np.float32)
    b7 = np.zeros(7, np.float32)
    for j in range(7):
        W7[:, j] = Ws[j // 4, :, j % 4]
        b7[j] = bs[j // 4, j % 4]
    W2 = np.zeros((128, 16), np.float32)
    W2[0:64, 0:7] = W7
    W2[64:128, 8:15] = W7
    W2 = W2.astype(ml_dtypes.bfloat16)

    with_bias = bool(np.any(b7 != 0.0))
    extra = {}
    if with_bias:
        ones2 = np.ones((128, 128), ml_dtypes.bfloat16)
        bias2 = np.zeros((128, 512), np.float32)
        for s in range(7):
            bias2[:, s::16] = b7[s] / 128.0
            bias2[:, 8 + s::16] = b7[s] / 128.0
        extra = {"ones2": ones2, "bias2": bias2.astype(ml_dtypes.bfloat16)}

    in_maps = []
    for c in range(N_CORES):
        xc = x[c * BC:(c + 1) * BC]
        xT2 = np.ascontiguousarray(
            xc.reshape(2, HALF, D).transpose(0, 2, 1).reshape(128, HALF)
        ).astype(ml_dtypes.bfloat16)
        in_maps.append({"xT2": xT2, "w2": W2, **extra})
    return in_maps, with_bias


def _gather_output(results):
    """Invert the device layout; see module docstring for the index map."""
    outs = []
    for c in range(N_CORES):
        yraw = np.asarray(results[c]["y"], dtype=np.float32)
        yc = (
            yraw.reshape(NQ, 128, 4, 2, 32)    # q m tq h i
            .transpose(3, 0, 2, 4, 1)          # h q tq i m
            .reshape(BC)
        )
        outs.append(yc)
    return np.concatenate(outs).astype(np.float32)


def run(inputs, trace=False, **run_kwargs):
    """Build, execute on 8 cores, and gather. Returns (y, BassKernelResults)."""
    from concourse.bass_utils import run_bass_kernel_spmd

    in_maps, with_bias = _prep_inputs(inputs["x"], inputs["Ws"], inputs["bs"])
    nc = build_program(with_bias=with_bias)
    res = run_bass_kernel_spmd(
        nc, in_maps, core_ids=list(range(N_CORES)), trace=trace, **run_kwargs
    )
    return _gather_output(res.results), res


def kernel(x, Ws, bs):
    y, _ = run({"x": x, "Ws": Ws, "bs": bs})
    return y


if __name__ == "__main__":
    rng = np.random.default_rng(0)
    x = rng.standard_normal((B_TOTAL, D), dtype=np.float32)
    Ws = (rng.standard_normal((4, 64, 4)) * 0.1).astype(np.float32)
    bs = np.zeros((4, 4), np.float32)
    y = kernel(x, Ws, bs)
    print("kernel ran, y:", y.shape, y.dtype, y[:4])
